# revision 11
# baseline (speedup 1.0000x reference)
"""Contrastive-learning loss kernel for 8 Trainium2 NeuronCores.

Strategy (data parallel): core c owns rows [c*1024, (c+1)*1024). Inputs are
rotated on the host (np.roll) so every core runs the *same* program with its
local rows at positions 0..1023. Each core:
  - L2-normalizes all B rows, transposes to [D, B] bf16 (PE transposes),
  - computes its [1024, B] similarity block with PE matmuls (K=D=128),
  - exp(10*sim - 10) on ACT with fused row-sum accumulation (all_sum),
  - masked positive sums via one fused DVE scalar_tensor_tensor per row tile,
  - validity counts via a 256-bin label histogram + tiny matmul gather,
  - per-row loss = log(all) - log(pos), masked partial sums -> [1, 2] output.
Host sums the 8 partial (loss_sum, n_valid) pairs and forms the masked mean.

Note: max-subtraction in the reference cancels exactly in pos/all, so a
constant shift of 10 (= max possible sim = 1/temperature) is used instead.
"""

import numpy as np
from contextlib import ExitStack

B, D = 8192, 128
N_CORES = 8
P = 128
LOCAL = B // N_CORES          # 1024 rows per core
RT = LOCAL // P               # 8 local row tiles
NT = B // P                   # 64 column tiles
ISCALE = 10.0                 # 1 / TEMPERATURE
EPS = 1e-8

_CACHED_NC = None


def build_nc():
    import concourse.bacc as bacc
    import concourse.mybir as mybir
    import concourse.tile as tile
    from concourse.masks import make_identity

    dt = mybir.dt
    f32, bf16, i32 = dt.float32, dt.bfloat16, dt.int32
    Alu = mybir.AluOpType
    Act = mybir.ActivationFunctionType
    AX = mybir.AxisListType

    nc = bacc.Bacc("TRN2", target_bir_lowering=False, debug=False,
                   num_devices=N_CORES)
    emb = nc.dram_tensor("emb", [B, D], f32, kind="ExternalInput")
    lab = nc.dram_tensor("lab", [B], f32, kind="ExternalInput")
    out = nc.dram_tensor("partial", [1, 2], f32, kind="ExternalOutput")

    with tile.TileContext(nc) as tc, ExitStack() as ctx:
        singles = ctx.enter_context(tc.tile_pool(name="singles", bufs=1))

        # persistent tensors
        lab_bcast = singles.tile([P, B], bf16)   # lab[j] replicated across partitions
        embT = singles.tile([P, B], bf16)        # normalized, transposed embeddings
        lab_sb = singles.tile([P, RT], f32)      # lab_sb[p, t] = lab[t*128 + p]
        ident = singles.tile([P, P], bf16)
        iota_row = singles.tile([P, P], bf16)   # iota_row[p, q] = q
        iota_f = singles.tile([P, 1], f32)
        iota_hi = singles.tile([P, 1], f32)
        hist = singles.tile([P, 2], f32)
        hist_bf = singles.tile([P, 2], bf16)
        count = singles.tile([P, RT], f32)
        all_acc = singles.tile([P, RT, 4], f32)
        pos_raw = singles.tile([P, RT], f32)
        dvec = singles.tile([P, RT], f32)
        rnorm = singles.tile([P, NT], f32)
        bias_m10 = singles.tile([P, 1], f32)
        bias_0 = singles.tile([P, 1], f32)

        with tc.tile_pool(name="ph0", bufs=1) as ph0, \
             tc.tile_pool(name="ph0s", bufs=2) as ph0s:
            # ---- loads ----
            e_rows = ph0.tile([P, NT, D], f32)   # e_rows[p, r, :] = emb[r*128+p, :]
            nc.sync.dma_start(out=e_rows[:],
                              in_=emb.ap().rearrange("(r p) d -> p r d", p=P))
            nc.sync.dma_start(out=lab_sb[:],
                              in_=lab.ap().rearrange("(r p) -> p r", p=P)[:, 0:RT])
            # broadcast labels across partitions, casting f32 -> bf16 (SWDGE)
            nc.gpsimd.dma_start(out=lab_bcast[:],
                                in_=lab.ap().partition_broadcast(P))
            make_identity(nc, ident[:])
            nc.vector.memset(bias_m10[:], -ISCALE)
            nc.vector.memset(bias_0[:], 0.0)
            iota_i = ph0.tile([P, 1], i32)
            nc.gpsimd.iota(iota_i[:], pattern=[[0, 1]], base=0, channel_multiplier=1)
            nc.vector.tensor_copy(iota_f[:], iota_i[:])
            nc.vector.tensor_scalar_add(iota_hi[:], iota_f[:], 128.0)
            iota_ri = ph0.tile([P, P], i32)
            nc.gpsimd.iota(iota_ri[:], pattern=[[1, P]], base=0,
                           channel_multiplier=0)
            nc.vector.tensor_copy(iota_row[:], iota_ri[:])

            # ---- row norms: n2[p, r] = sum_d e_rows[p, r, d]^2 ----
            sq = ph0.tile([P, NT, D], f32)
            nc.gpsimd.tensor_mul(sq[:], e_rows[:], e_rows[:])
            n2 = ph0.tile([P, NT], f32)
            nc.vector.tensor_reduce(n2[:], sq[:], axis=AX.X, op=Alu.add)

            # ---- rnorm = 1/sqrt(n2): bit-trick seed + 2 Newton steps ----
            t0 = ph0.tile([P, NT], i32)
            nc.vector.tensor_scalar(out=t0[:], in0=n2[:].bitcast(i32),
                                    scalar1=1, scalar2=None,
                                    op0=Alu.logical_shift_right)
            y0i = ph0.tile([P, NT], i32)
            nc.vector.tensor_scalar(out=y0i[:], in0=t0[:],
                                    scalar1=-1, scalar2=0x5F3759DF,
                                    op0=Alu.mult, op1=Alu.add)
            yy = ph0.tile([P, NT], f32)
            tt_ = ph0.tile([P, NT], f32)
            cur = y0i[:].bitcast(f32)
            for _ in range(2):
                nc.vector.tensor_mul(yy[:], cur, cur)
                nc.vector.tensor_mul(tt_[:], n2[:], yy[:])
                nc.vector.tensor_scalar(out=tt_[:], in0=tt_[:],
                                        scalar1=-0.5, scalar2=1.5,
                                        op0=Alu.mult, op1=Alu.add)
                nc.vector.tensor_mul(rnorm[:], cur, tt_[:])
                cur = rnorm[:]

            # ---- normalize (f32) and cast to bf16 ----
            e_n = ph0.tile([P, NT, D], bf16)
            for r in range(NT):
                nc.vector.tensor_scalar_mul(e_n[:, r, :], e_rows[:, r, :],
                                            rnorm[:, r:r + 1])

            # ---- transpose to embT via PE, 4 tiles per PSUM bank ----
            with tc.tile_pool(name="tp_psum", bufs=4, space="PSUM") as tpp:
                for grp in range(NT // 4):
                    pt = tpp.tile([P, 4, P], bf16)
                    for k in range(4):
                        nc.tensor.transpose(pt[:, k, :], e_n[:, grp * 4 + k, :],
                                            ident[:])
                    nc.vector.tensor_copy(embT[:, grp * 512:(grp + 1) * 512], pt[:])

            # ---- label histogram: hist[c, g] = #{j : lab[j] == c + 128 g} ----
            for g2 in range(2):
                scrP = ph0s.tile([P, B], bf16, tag="scrP")
                nc.vector.tensor_scalar(out=scrP[:], in0=lab_bcast[:],
                                        scalar1=(iota_f if g2 == 0 else iota_hi)[:],
                                        scalar2=None, op0=Alu.is_equal,
                                        op1=Alu.add,
                                        accum_out=hist[:, g2:g2 + 1])
            nc.vector.tensor_copy(hist_bf[:], hist[:])

            # ---- per-local-row count = hist[lab_i] via one-hot matmul ----
            with tc.tile_pool(name="cnt_psum", bufs=2, space="PSUM") as cpool:
                for t in range(RT):
                    cp = cpool.tile([P, 1], f32)
                    for g2 in range(2):
                        Pl = ph0s.tile([P, P], bf16, tag="Pl")
                        nc.vector.tensor_scalar(
                            out=Pl[:], in0=lab_bcast[:, t * P:(t + 1) * P],
                            scalar1=(iota_f if g2 == 0 else iota_hi)[:],
                            scalar2=None, op0=Alu.is_equal)
                        nc.tensor.matmul(cp[:], Pl[:], hist_bf[:, g2:g2 + 1],
                                         start=(g2 == 0), stop=(g2 == 1))
                    nc.vector.tensor_copy(count[:, t:t + 1], cp[:])

        # ---- main loop: per local row tile, [128, B] similarity block ----
        with tc.tile_pool(name="mm_psum", bufs=2, space="PSUM") as mpool, \
             tc.tile_pool(name="exp_pool", bufs=2) as epool, \
             tc.tile_pool(name="scr_pool", bufs=2) as spool:
            for t in range(RT):
                lhsT = embT[:, t * P:(t + 1) * P]
                exp_sb = epool.tile([P, B], bf16, tag="exp")
                for g in range(4):
                    ps = mpool.tile([P, 2048], f32, tag="ps")
                    for c in range(4):
                        nc.tensor.matmul(
                            ps[:, c * 512:(c + 1) * 512], lhsT,
                            embT[:, g * 2048 + c * 512: g * 2048 + (c + 1) * 512],
                            start=True, stop=True)
                    nc.scalar.activation(out=exp_sb[:, g * 2048:(g + 1) * 2048],
                                         in_=ps[:], func=Act.Exp,
                                         bias=bias_m10[:], scale=ISCALE,
                                         accum_out=all_acc[:, t, g:g + 1])
                scr = spool.tile([P, B], bf16, tag="scr")
                nc.vector.scalar_tensor_tensor(
                    out=scr[:], in0=lab_bcast[:], scalar=lab_sb[:, t:t + 1],
                    in1=exp_sb[:], op0=Alu.is_equal, op1=Alu.mult,
                    accum_out=pos_raw[:, t:t + 1])
                scr2 = spool.tile([P, P], bf16, tag="scr2")
                nc.vector.scalar_tensor_tensor(
                    out=scr2[:], in0=iota_row[:], scalar=iota_f[:],
                    in1=exp_sb[:, t * P:(t + 1) * P],
                    op0=Alu.is_equal, op1=Alu.mult,
                    accum_out=dvec[:, t:t + 1])

        # ---- finals ----
        all_raw = singles.tile([P, RT], f32)
        nc.vector.tensor_reduce(all_raw[:], all_acc[:], axis=AX.X, op=Alu.add)
        pos = singles.tile([P, RT], f32)
        nc.vector.tensor_sub(pos[:], pos_raw[:], dvec[:])
        allv = singles.tile([P, RT], f32)
        nc.vector.tensor_sub(allv[:], all_raw[:], dvec[:])
        nc.vector.tensor_scalar_max(allv[:], allv[:], EPS)
        inv = singles.tile([P, RT], f32)
        nc.vector.reciprocal(inv[:], allv[:])
        ratio = singles.tile([P, RT], f32)
        nc.vector.tensor_mul(ratio[:], pos[:], inv[:])
        nc.vector.tensor_scalar_max(ratio[:], ratio[:], EPS)
        lg = singles.tile([P, RT], f32)
        nc.scalar.activation(out=lg[:], in_=ratio[:], func=Act.Ln,
                             bias=bias_0[:])
        valid = singles.tile([P, RT], f32)
        nc.vector.tensor_scalar(out=valid[:], in0=count[:], scalar1=1.5,
                                scalar2=None, op0=Alu.is_gt)
        lv = singles.tile([P, RT], f32)
        nc.vector.tensor_mul(lv[:], lg[:], valid[:])
        fin2 = singles.tile([P, 2], f32)
        nc.vector.tensor_reduce(fin2[:, 0:1], lv[:], axis=AX.X, op=Alu.add)
        nc.vector.tensor_reduce(fin2[:, 1:2], valid[:], axis=AX.X, op=Alu.add)
        ones = singles.tile([P, 1], f32)
        nc.vector.memset(ones[:], 1.0)
        with tc.tile_pool(name="fin_psum", bufs=1, space="PSUM") as fpool:
            fp = fpool.tile([1, 2], f32)
            nc.tensor.matmul(fp[:], ones[:], fin2[:], start=True, stop=True)
            osb = singles.tile([1, 2], f32)
            nc.vector.tensor_copy(osb[:], fp[:])
            nc.sync.dma_start(out=out[:, :], in_=osb[:])

    nc.compile()
    return nc


def get_nc():
    global _CACHED_NC
    if _CACHED_NC is None:
        _CACHED_NC = build_nc()
    return _CACHED_NC


def make_in_maps(embeddings, labels):
    emb = np.ascontiguousarray(np.asarray(embeddings, dtype=np.float32))
    lab_f = np.asarray(labels).astype(np.float32)
    in_maps = []
    for c in range(N_CORES):
        sh = c * LOCAL
        in_maps.append({
            "emb": np.ascontiguousarray(np.roll(emb, -sh, axis=0)),
            "lab": np.ascontiguousarray(np.roll(lab_f, -sh)),
        })
    return in_maps


def finish(results):
    ls = 0.0
    nv = 0.0
    for r in results:
        p = np.asarray(r["partial"], dtype=np.float64).reshape(-1)
        ls += float(p[0])
        nv += float(p[1])
    val = (-ls / max(nv, 1.0)) if nv > 0 else 0.0
    return np.array(val, dtype=np.float32)


def kernel(embeddings, labels):
    from concourse.bass_utils import run_bass_kernel_spmd
    nc = get_nc()
    in_maps = make_in_maps(embeddings, labels)
    res = run_bass_kernel_spmd(nc, in_maps, list(range(N_CORES)))
    return finish(res.results)


if __name__ == "__main__":
    rng = np.random.default_rng(0)
    e = rng.standard_normal((B, D)).astype(np.float32)
    l = rng.integers(0, 256, size=(B,)).astype(np.int64)
    print(kernel(e, l))
